# revision 16
# baseline (speedup 1.0000x reference)
"""Self-attention (QKV proj + softmax(QK^T/s)V) on TRN2, 8 NeuronCores.

Sharding: data-parallel over batch (B=4) x 2-way sequence-parallel over
queries -> 8 shards of 2048 query rows.  Each core computes K/V for its
full batch sequence (N=4096) and attention output for its query half.

Kernel strategy (per core), flash-attention style with NO HBM score
materialization:
  - All matmul operands bf16 (fp32 PSUM accumulation).
  - Projections computed transposed: QT[e,m] / KT[e,n] via
    out = (W^T)^T.T... i.e. lhsT = WqT chunk [d,e], rhs = xT [d,m].
    V kept natural [n,e]: lhsT = xT chunk [d,n], rhs = WvT [d,e].
  - Scores computed transposed: ST[n,m] = lhsT(KT).T @ rhs(QT) so the
    softmax reduction (over n) is the matmul contraction dim of PV.
  - exp on ACT without max-subtraction (scores ~N(0,1): overflow-safe).
  - Denominator for free: V is extended with a ones column (e'=257);
    O'[m,0:256] = sum_n expST*V, O'[m,256] = row sum of exp.
  - Epilogue: per-partition multiply by 1/O'[:,256], add bv, DMA out.
  - 1/scale and bq/scale folded into Wq/bq on host.
"""

import numpy as np
import ml_dtypes
from contextlib import ExitStack

import concourse.bass as bass
import concourse.tile as tile
from concourse import bacc, mybir
from concourse.bass_utils import run_bass_kernel_spmd

B, N, D = 4, 4096, 256
NCORES = 8
MQ = (B * N) // NCORES  # 2048 query rows per core

BF16 = mybir.dt.bfloat16
F32 = mybir.dt.float32
F32R = mybir.dt.float32r
NPBF16 = ml_dtypes.bfloat16

# matmul operand precision: "bf16" or "f32r"
import os as _os

MM_DTYPE = _os.environ.get("KERNEL_MM_DTYPE", "bf16")

ALU = mybir.AluOpType
ACTF = mybir.ActivationFunctionType


def build_program(seq=N, mq=MQ, mm_dtype=None):
    """One SPMD program; per-core behavior differs only through input data."""
    mm_dtype = mm_dtype or MM_DTYPE
    # float32r must be declared end-to-end (producers round on write).
    XDT = F32R if mm_dtype == "f32r" else BF16

    def mo(ap):
        return ap

    # PV moving operand width: D values + ones column; fp32 streaming
    # requires an even element count, so pad to 258 for f32r.
    ve = D + 2 if mm_dtype == "f32r" else D + 1
    nchunk = seq // 128          # key chunks of 128
    m_group = min(512, mq)       # query columns processed per ST pass
    ngroup = mq // m_group
    nsub = m_group // 128        # 128-row output subtiles per group
    ndc = D // 128               # contraction (d) chunks

    nc = bacc.Bacc("TRN2", debug=False)

    xt = nc.dram_tensor("xt", [D, seq], XDT, kind="ExternalInput").ap()
    xqt = nc.dram_tensor("xqt", [D, mq], XDT, kind="ExternalInput").ap()
    # w = [WqT/s | WkT | WvT] packed column-wise -> one big DMA per d-chunk
    w = nc.dram_tensor("w", [D, 3 * D], XDT, kind="ExternalInput").ap()
    # bqk = [bq/s ; bk] packed -> one DMA
    bqk = nc.dram_tensor("bqk", [2 * D], F32, kind="ExternalInput").ap()
    bv = nc.dram_tensor("bv", [D], F32, kind="ExternalInput").ap()
    out = nc.dram_tensor("out", [mq, D], F32, kind="ExternalOutput").ap()

    with tile.TileContext(nc) as tc, ExitStack() as ctx:
        singles = ctx.enter_context(tc.tile_pool(name="singles", bufs=1))
        st_psum = ctx.enter_context(
            tc.tile_pool(name="st_psum", bufs=4, space="PSUM")
        )
        o_psum = ctx.enter_context(
            tc.tile_pool(name="o_psum", bufs=1, space="PSUM")
        )
        expp = ctx.enter_context(tc.tile_pool(name="expp", bufs=4))
        outp = ctx.enter_context(tc.tile_pool(name="outp", bufs=3))

        # ---- constants in ----
        def named(pool, shape, dtype, nm):
            return pool.tile(shape, dtype, name=nm, tag=nm)

        # DMA order: packed weights first (first matmuls need them), then
        # x chunks interleaved so projections can start after ~256KB.
        # Transfers within a queue serialize; spread the two d-chunks
        # across the HWDGE (sync) and SWDGE (gpsimd) queues.
        dmae = [nc.sync, nc.gpsimd]
        w_sb = []
        for dc in range(ndc):
            t = named(singles, [128, 3 * D], XDT, f"w{dc}")
            dmae[dc % 2].dma_start(out=t, in_=w[dc * 128 : (dc + 1) * 128, :])
            w_sb.append(t)

        def wsl(key, dc, ec=None):
            base = {"wq": 0, "wk": D, "wv": 2 * D}[key]
            if ec is None:
                return w_sb[dc][:, base : base + D]
            return w_sb[dc][:, base + ec * 128 : base + (ec + 1) * 128]

        # TensorScalarPtr supports a single sync-wait, and the fused
        # bias-add copies already wait on PE; stage the biases through a
        # DVE copy so their dependency is same-engine (program order).
        b_stage = named(singles, [128, 2 * ndc], F32, "b_stage")
        nc.sync.dma_start(
            out=b_stage,
            in_=bass.AP(
                tensor=bqk.tensor, offset=bqk.offset, ap=[[1, 128], [128, 2 * ndc]]
            ),
        )
        bvb = named(singles, [128, D], F32, "bvb")
        nc.sync.dma_start(
            out=bvb,
            in_=bass.AP(tensor=bv.tensor, offset=bv.offset, ap=[[0, 128]] + bv.ap),
        )

        nxc = max(1, seq // 1024)   # x DMA chunks of 1024 columns
        xcw = seq // nxc
        qcw = mq // nxc
        xt_sb = [named(singles, [128, seq], XDT, f"xt{dc}") for dc in range(ndc)]
        xq_sb = [named(singles, [128, mq], XDT, f"xq{dc}") for dc in range(ndc)]
        for c in range(nxc):
            for dc in range(ndc):
                sl = slice(c * qcw, (c + 1) * qcw)
                nc.sync.dma_start(
                    out=xq_sb[dc][:, sl], in_=xqt[dc * 128 : (dc + 1) * 128, sl]
                )
            for dc in range(ndc):
                sl = slice(c * xcw, (c + 1) * xcw)
                nc.sync.dma_start(
                    out=xt_sb[dc][:, sl], in_=xt[dc * 128 : (dc + 1) * 128, sl]
                )

        bqt = named(singles, [128, 2 * ndc], F32, "bqt")
        nc.vector.tensor_copy(out=bqt, in_=b_stage)

        # ---- projections ----
        qts = [named(singles, [128, mq], XDT, f"qts{ec}") for ec in range(ndc)]
        kts = [named(singles, [128, seq], XDT, f"kts{ec}") for ec in range(ndc)]
        vp = named(singles, [128, nchunk, ve], XDT, "vp")
        ones_col = vp[:, :, D:ve]
        if XDT == F32R:
            # MEMSET has no float32r encoding; write the bits as float32.
            ones_col = ones_col.bitcast(F32)
        nc.vector.memset(ones_col, 1.0)

        def project_t(dst, w_key, src_sb, width, bias_col, ec, mc):
            # dst[e 128, width] += sum_dc w[dc][:, e].T @ src[dc][:, mc]
            ps = st_psum.tile([128, 512], F32, tag="st", name="ps_proj")
            sl = slice(mc * width, (mc + 1) * width)
            for dc in range(ndc):
                nc.tensor.matmul(
                    ps[:, :width],
                    lhsT=mo(wsl(w_key, dc, ec)),
                    rhs=mo(src_sb[dc][:, sl]),
                    start=(dc == 0),
                    stop=(dc == ndc - 1),
                )
            nc.vector.tensor_scalar(
                out=dst[:, sl],
                in0=ps[:, :width],
                scalar1=bqt[:, bias_col : bias_col + 1],
                scalar2=None,
                op0=ALU.add,
            )

        # Emit projections in x-column order so PE work becomes ready in
        # DMA arrival order.  V copies go to ScalarE (ACT is idle here,
        # DVE carries the fused bias-adds).
        qw = min(512, mq)
        kw = min(512, seq)
        for c in range(nxc):
            for mc in range(c * qcw // qw, (c + 1) * qcw // qw):
                for ec in range(ndc):
                    project_t(qts[ec], "wq", xq_sb, qw, ec, ec, mc)
            for mc in range(c * xcw // kw, (c + 1) * xcw // kw):
                for ec in range(ndc):
                    project_t(kts[ec], "wk", xt_sb, kw, ndc + ec, ec, mc)
            for j in range(c * xcw // 128, (c + 1) * xcw // 128):
                ps = st_psum.tile([128, 512], F32, tag="st", name="ps_v")
                for dc in range(ndc):
                    nc.tensor.matmul(
                        ps[:, :D],
                        lhsT=mo(xt_sb[dc][:, j * 128 : (j + 1) * 128]),
                        rhs=mo(wsl("wv", dc)),
                        start=(dc == 0),
                        stop=(dc == ndc - 1),
                    )
                nc.scalar.activation(out=vp[:, j, 0:D], in_=ps[:, :D], func=ACTF.Copy)

        # ---- main attention loop ----
        LOOKAHEAD = 2
        for g in range(ngroup):
            m0 = g * m_group
            o_tiles = [
                o_psum.tile([128, ve], F32, tag=f"o{s}", name=f"o{s}")
                for s in range(nsub)
            ]
            pending = {}
            for t in range(nchunk + LOOKAHEAD):
                if t < nchunk:
                    j = t
                    ps = st_psum.tile([128, 512], F32, tag="st", name="ps_st")
                    for dc in range(ndc):
                        nc.tensor.matmul(
                            ps[:, :m_group],
                            lhsT=mo(kts[dc][:, j * 128 : (j + 1) * 128]),
                            rhs=mo(qts[dc][:, m0 : m0 + m_group]),
                            start=(dc == 0),
                            stop=(dc == ndc - 1),
                        )
                    ex = expp.tile([128, m_group], XDT, tag="ex", name="ex")
                    nc.scalar.activation(out=ex, in_=ps[:, :m_group], func=ACTF.Exp)
                    pending[j] = ex
                if t >= LOOKAHEAD:
                    j = t - LOOKAHEAD
                    ex = pending.pop(j)
                    for s in range(nsub):
                        nc.tensor.matmul(
                            o_tiles[s],
                            lhsT=mo(ex[:, s * 128 : (s + 1) * 128]),
                            rhs=mo(vp[:, j, :]),
                            start=(j == 0),
                            stop=(j == nchunk - 1),
                        )
            for s in range(nsub):
                ob = outp.tile([128, D], F32, tag="ob", name="ob")
                rc = outp.tile([128, 1], F32, tag="rc", name="rc")
                nc.vector.reciprocal(rc, o_tiles[s][:, D : D + 1])
                nc.vector.tensor_scalar(
                    out=ob,
                    in0=o_tiles[s][:, 0:D],
                    scalar1=rc,
                    scalar2=None,
                    op0=ALU.mult,
                )
                nc.vector.tensor_add(ob, ob, bvb)
                r0 = (g * nsub + s) * 128
                nc.sync.dma_start(out=out[r0 : r0 + 128, :], in_=ob)

    nc.compile()
    return nc


_NC_CACHE = {}


def _get_nc(seq=N, mq=MQ):
    key = (seq, mq, MM_DTYPE)
    if key not in _NC_CACHE:
        _NC_CACHE[key] = build_program(seq, mq)
    return _NC_CACHE[key]


def make_in_maps(x, Wq, bq, Wk, bk, Wv, bv, scale):
    s = float(np.asarray(scale, np.float32).reshape(-1)[0])
    wq_t = np.asarray(Wq, np.float32).T / s
    wk_t = np.asarray(Wk, np.float32).T
    wv_t = np.asarray(Wv, np.float32).T
    npxdt = np.float32 if MM_DTYPE == "f32r" else NPBF16
    w_all = np.ascontiguousarray(
        np.concatenate([wq_t, wk_t, wv_t], axis=1)
    ).astype(npxdt)  # [D, 3D]
    bqk = np.concatenate(
        [np.asarray(bq, np.float32) / s, np.asarray(bk, np.float32)]
    )
    bv_f = np.asarray(bv, np.float32)
    xtb = np.ascontiguousarray(
        np.asarray(x, np.float32).transpose(0, 2, 1)
    ).astype(npxdt)  # [B, D, N]
    half = MQ
    in_maps = []
    for c in range(NCORES):
        b, h = divmod(c, NCORES // B)
        in_maps.append(
            {
                "xt": xtb[b],
                "xqt": np.ascontiguousarray(xtb[b][:, h * half : (h + 1) * half]),
                "w": w_all,
                "bqk": bqk,
                "bv": bv_f,
            }
        )
    return in_maps


def _install_ntff_hook():
    """Register the axon NTFF profile hook if the image's antenv lacks it."""
    import sys
    import types

    try:
        from antenv.axon_hooks import get_axon_ntff_profile_hook  # noqa: F401

        return
    except ImportError:
        pass
    mod = types.ModuleType("antenv.axon_hooks")
    holder = {"h": None}
    mod.set_axon_ntff_profile_hook = lambda h: holder.__setitem__("h", h)
    mod.get_axon_ntff_profile_hook = lambda: holder["h"]
    sys.modules["antenv.axon_hooks"] = mod
    import antenv

    antenv.axon_hooks = mod
    try:
        from trn_agent_boot.trn_boot import _ntff_profile_via_ctypes

        mod.set_axon_ntff_profile_hook(
            _ntff_profile_via_ctypes("/opt/axon/libaxon_pjrt.so")
        )
    except Exception:
        pass


def _run(inputs, trace=False, **kw):
    if trace:
        _install_ntff_hook()
    nc = _get_nc()
    in_maps = make_in_maps(**inputs)
    res = run_bass_kernel_spmd(nc, in_maps, list(range(NCORES)), trace=trace, **kw)
    out = np.empty((B, N, D), np.float32)
    for c in range(NCORES):
        b, h = divmod(c, NCORES // B)
        out[b, h * MQ : (h + 1) * MQ, :] = res.results[c]["out"]
    return out, res


def kernel(**inputs) -> np.ndarray:
    out, _ = _run(inputs)
    return out
